# revision 2
# baseline (speedup 1.0000x reference)
"""Trainium2 Bass kernel for nn_LinearNNEncoder (fused Linear+GELU, masked per-batch
mean/std over ragged sequences), data-parallel over 8 NeuronCores.

Contract: kernel(**inputs) takes the FULL inputs (x [64,2048,300] f32, W [300,300],
b [300]) and returns the FULL output [64, 600] f32 (concat(std, mean) per batch).

Strategy per core (8 batches of 2048 tokens each):
  - x is host-padded to D=384 with a ones-column at d=300 (folds the bias into the
    GEMM) and zeros beyond; W^T is host-packed into 3 K-tiles of 128 with b as the
    bias row.
  - Per 128-token tile: DMA load -> PE transpose (3x 128x128, via identity) ->
    one ACT copy PSUM->SBUF -> 3 accumulating fp32r matmuls (y = x @ W^T + b) ->
    ACT exact-GELU -> DVE padding mask (row == all -1.0) -> DVE y^2 ->
    PE stats matmuls with the mask vector as stationary weights
    (masked sum(y) | count | masked sum(y^2) accumulate in PSUM over the batch).
  - Per batch: drain stats; final epilogue computes mean/std for all 8 batches at
    once and DMAs out [8, 600].
All tensors fp32 in DRAM; matmuls run as float32r (fp32 storage, ~fp22 multiply,
full PE rate at N>=256).
"""
import numpy as np

B, T, D = 64, 2048, 300
NCORES = 8
B_LOC = B // NCORES     # batches per core
DP = 384                # padded depth: 3 K-tiles of 128 (col 300 = ones, rest 0)
TPB = T // 128          # token tiles per batch
KT = 3

_cache = {}


def _build_nc(b_loc=B_LOC, tpb=TPB):
    from contextlib import ExitStack
    import concourse.tile as tile
    from concourse import mybir, bacc

    f32 = mybir.dt.float32
    f32r = mybir.dt.float32r
    AF = mybir.ActivationFunctionType
    OP = mybir.AluOpType

    nt = b_loc * tpb
    P = 128

    nc = bacc.Bacc("TRN2", target_bir_lowering=False, debug=False)
    x_dram = nc.dram_tensor("x", [nt, P, DP], f32r, kind="ExternalInput")
    wt_dram = nc.dram_tensor("wt", [KT, P, D + 2], f32r, kind="ExternalInput")
    id_dram = nc.dram_tensor("ident", [P, P], f32r, kind="ExternalInput")
    out_dram = nc.dram_tensor("out", [b_loc, 2 * D], f32, kind="ExternalOutput")

    with ExitStack() as ctx:
        tc = ctx.enter_context(tile.TileContext(nc))
        const = ctx.enter_context(tc.tile_pool(name="const", bufs=1))
        xp = ctx.enter_context(tc.tile_pool(name="xp", bufs=4))
        xmp = ctx.enter_context(tc.tile_pool(name="xmp", bufs=4))
        xtp = ctx.enter_context(tc.tile_pool(name="xtp", bufs=3))
        yp = ctx.enter_context(tc.tile_pool(name="yp", bufs=3))
        y2p = ctx.enter_context(tc.tile_pool(name="y2p", bufs=3))
        eqp = ctx.enter_context(tc.tile_pool(name="eqp", bufs=2))
        cntp = ctx.enter_context(tc.tile_pool(name="cntp", bufs=3))
        mmp = ctx.enter_context(tc.tile_pool(name="mmp", bufs=3))
        epil = ctx.enter_context(tc.tile_pool(name="epil", bufs=1))
        drp = ctx.enter_context(tc.tile_pool(name="drp", bufs=2))
        ps_xt = ctx.enter_context(tc.tile_pool(name="ps_xt", bufs=3, space="PSUM"))
        ps_w = ctx.enter_context(tc.tile_pool(name="ps_w", bufs=1, space="PSUM"))
        ps_y = ctx.enter_context(tc.tile_pool(name="ps_y", bufs=2, space="PSUM"))
        ps_s = ctx.enter_context(tc.tile_pool(name="ps_s", bufs=1, space="PSUM"))
        ps_q = ctx.enter_context(tc.tile_pool(name="ps_q", bufs=1, space="PSUM"))

        wt_sb = const.tile([P, KT, D + 2], f32r)
        nc.sync.dma_start(wt_sb[:], wt_dram.ap().rearrange("k p o -> p k o"))
        ident = const.tile([P, P], f32r)
        nc.sync.dma_start(ident[:], id_dram.ap())
        sums_all = const.tile([b_loc, D + 4], f32)   # [:, 0:300]=sum(masked y), [:, 300]=n
        sumsq_all = const.tile([b_loc, D], f32)
        out_sb = const.tile([b_loc, 2 * D], f32)

        pxtw = ps_w.tile([P, 2 * P], f32r, name="pxtw", tag="warm")
        nc.tensor.transpose(pxtw[:, 0:P], ident[:], ident[:])
        nc.tensor.transpose(pxtw[:, P:2 * P], wt_sb[:, 0, 0:P], ident[:])

        stA = {}
        stB = {}
        cur = {}

        for step in range(nt + 2):
            # phase B (tile i-1): GEMM, GELU, mask, y^2  (emitted first so ACT-order
            # keeps each PE matmul at <=1 new semaphore wait)
            j = step - 1
            if 0 <= j < nt:
                xm_sb, xT_sb = stA.pop(j)
                py = ps_y.tile([P, D + 2], f32, name=f"py_{j}", tag="py")
                for k in range(KT):
                    nc.tensor.matmul(
                        py[:, 0:D + 2], xT_sb[:, k * P:(k + 1) * P], wt_sb[:, k, :],
                        start=(k == 0), stop=(k == KT - 1),
                    )
                y_sb = yp.tile([P, D + 2], f32r, name=f"y_{j}", tag="y")
                nc.scalar.activation(y_sb[:, 0:D + 2], py[:], AF.Gelu)

                eq = eqp.tile([P, 8], f32, name=f"eq_{j}", tag="eq")
                cnt = cntp.tile([P, 1], f32, name=f"cnt_{j}", tag="cnt")
                nc.vector.tensor_scalar(
                    eq[:], xm_sb[:], -1.0, None, OP.is_equal, OP.add,
                    accum_out=cnt[:],
                )
                m_sb = mmp.tile([P, 1], f32r, name=f"m_{j}", tag="m")
                nc.vector.tensor_scalar(m_sb[:], cnt[:], 8.0, None, OP.is_lt)

                y2_sb = y2p.tile([P, D], f32r, name=f"y2_{j}", tag="y2")
                nc.vector.tensor_mul(y2_sb[:], y_sb[:, 0:D], y_sb[:, 0:D])
                stB[j] = (y_sb, y2_sb, m_sb)

            # phase A (tile i): DMA load, PE transpose, ACT copy to SBUF
            i = step
            if i < nt:
                x_sb = xp.tile([P, DP], f32r, name=f"x_{i}", tag="x")
                nc.sync.dma_start(x_sb[:], x_dram.ap()[i])
                xm_sb = xmp.tile([P, 8], f32r, name=f"xm_{i}", tag="xm")
                nc.sync.dma_start(xm_sb[:], x_dram.ap()[i][:, 0:8])
                pxt = ps_xt.tile([P, DP], f32r, name=f"pxt_{i}", tag="pxt")
                for k in range(KT):
                    nc.tensor.transpose(
                        pxt[:, k * P:(k + 1) * P], x_sb[:, k * P:(k + 1) * P], ident[:]
                    )
                xT_sb = xtp.tile([P, DP], f32r, name=f"xt_{i}", tag="xt")
                nc.scalar.copy(xT_sb[:, 0:224], pxt[:, 0:224])
                nc.vector.tensor_copy(xT_sb[:, 224:DP], pxt[:, 224:DP])
                stA[i] = (xm_sb, xT_sb)

            # phase C (tile i-2): masked-stats matmuls, per-batch drain
            kk = step - 2
            if 0 <= kk < nt:
                b, jj = divmod(kk, tpb)
                if jj == 0:
                    cur["s"] = ps_s.tile([1, D + 4], f32, name=f"ps_s_{b}", tag="s")
                    cur["q"] = ps_q.tile([1, D], f32, name=f"ps_q_{b}", tag="q")
                y_sb, y2_sb, m_sb = stB.pop(kk)
                s_t, q_t = cur["s"], cur["q"]
                st = jj == 0
                sp = jj == tpb - 1
                nc.tensor.matmul(s_t[0:1, 0:D + 2], m_sb[:], y_sb[:, 0:D + 2], start=st, stop=sp)
                nc.tensor.matmul(q_t[0:1, 0:D], m_sb[:], y2_sb[:], start=st, stop=sp)
                if sp:
                    sdr = drp.tile([1, D + 4], f32, name=f"sdr_{b}", tag="sdr")
                    qdr = drp.tile([1, D + 4], f32, name=f"qdr_{b}", tag="qdr")
                    nc.vector.tensor_copy(sdr[0:1, 0:D + 1], s_t[0:1, 0:D + 1])
                    nc.scalar.copy(qdr[0:1, 0:D], q_t[0:1, 0:D])
                    nc.sync.dma_start(sums_all[b:b + 1, 0:D + 1], sdr[0:1, 0:D + 1])
                    nc.sync.dma_start(sumsq_all[b:b + 1, 0:D], qdr[0:1, 0:D])

        # epilogue: mean/std for all batches at once
        n_col = sums_all[:, D:D + 1]       # = 16 * n
        rn = epil.tile([b_loc, 1], f32)
        nc.vector.reciprocal(rn[:], n_col)  # = 1/(16 n)
        mean = epil.tile([b_loc, D], f32)
        nc.vector.tensor_scalar(mean[:], sums_all[:, 0:D], rn[:], 16.0, OP.mult, OP.mult)
        nc.scalar.copy(out_sb[:, D:2 * D], mean[:])
        nms16 = epil.tile([b_loc, D], f32)  # 16 * n * mean^2
        nc.vector.scalar_tensor_tensor(nms16[:], mean[:], n_col, mean[:], OP.mult, OP.mult)
        nm1 = epil.tile([b_loc, 1], f32)    # n - 1
        nc.vector.tensor_scalar(nm1[:], n_col, 0.0625, -1.0, OP.mult, OP.add)
        rnm1 = epil.tile([b_loc, 1], f32)
        nc.vector.reciprocal(rnm1[:], nm1[:])
        varn = epil.tile([b_loc, D], f32)   # sumsq - n*mean^2
        nc.vector.scalar_tensor_tensor(varn[:], nms16[:], -0.0625, sumsq_all[:, 0:D], OP.mult, OP.add)
        var2 = epil.tile([b_loc, D], f32)
        nc.vector.tensor_scalar(var2[:], varn[:], rnm1[:], 0.0, OP.mult, OP.max)
        nc.scalar.sqrt(out_sb[:, 0:D], var2[:])
        nc.sync.dma_start(out_dram.ap()[:], out_sb[:])

    nc.compile()
    return nc


def _prep_inputs(x, W, b):
    """Host prep: pad x with ones/zeros columns, pack W^T k-tiles with bias row."""
    x = np.ascontiguousarray(x, np.float32)
    xp = np.empty((B, T, DP), np.float32)
    xp[:, :, :D] = x
    xp[:, :, D] = 1.0
    xp[:, :, D + 1:] = 0.0
    shards = [
        xp[c * B_LOC:(c + 1) * B_LOC].reshape(B_LOC * T // 128, 128, DP)
        for c in range(NCORES)
    ]
    wt = np.zeros((KT * 128, D + 2), np.float32)
    wt[:D, :D] = np.asarray(W, np.float32).T
    wt[D, :D] = np.asarray(b, np.float32)
    wt[D, D] = 16.0
    wt = wt.reshape(KT, 128, D + 2)
    return shards, wt


def _trace_in_maps(ins):
    shards, wt = _prep_inputs(**ins)
    ident = np.eye(128, dtype=np.float32)
    return [{"x": shards[c], "wt": wt, "ident": ident} for c in range(NCORES)]


def kernel(x, W, b):
    from concourse.bass_utils import run_bass_kernel_spmd

    if "nc" not in _cache:
        _cache["nc"] = _build_nc()
    nc = _cache["nc"]

    shards, wt = _prep_inputs(x, W, b)
    ident = np.eye(128, dtype=np.float32)
    in_maps = [{"x": shards[c], "wt": wt, "ident": ident} for c in range(NCORES)]
    res = run_bass_kernel_spmd(nc, in_maps, core_ids=list(range(NCORES)))
    out = np.concatenate([res.results[c]["out"] for c in range(NCORES)], axis=0)
    return out.astype(np.float32)



# revision 3
# speedup vs baseline: 3.1017x; 3.1017x over previous
"""Trainium2 Bass kernel for nn_LinearNNEncoder (fused Linear+GELU, masked per-batch
mean/std over ragged sequences), data-parallel over 8 NeuronCores.

Contract: kernel(**inputs) takes FULL inputs (x [64,2048,300] f32, W [300,300],
b [300]) and returns the FULL output [64, 600] f32 (concat(std, mean) per batch).

Design (v3):
  - Host drops all-padding 128-token tiles (ragged lengths -> ~0.65x work), zeroes
    the remaining pad rows, and packs tiles into batch-aligned GROUPS of 1..4 tiles.
    Every core receives the same sorted multiset of group sizes (dummy all-zero
    groups pad each size class to a multiple of 8), so one SPMD program serves all
    cores while per-core data differs.
  - Per group: one dma_start_transpose loads x^T [128, 3, 128*sz] bf16 straight
    from DRAM (HW xbar transpose; d on partitions).  PE runs 9 matmuls (W-slice
    stationary [128,128] bf16, x^T moving) accumulating pre-activations per
    128-wide output slice in PSUM.  ScalarE applies exact GELU with the bias as a
    per-partition vector and emits the per-slice token-sum via accum_out (fp32
    datapath).  VectorE squares y (scalar_tensor_tensor) and emits sum(y^2) via
    accum_out.  Sums land in per-(slice, group) slots; one DMA returns them.
  - Host epilogue: sums slots per batch, subtracts the analytic contribution of
    the zeroed pad rows (gelu(b) per token; bf16-rounded for the square sum), and
    computes mean/std (unbiased, n<=1 and NaN edge cases per the reference).
"""
import numpy as np
import ml_dtypes

B, T, D = 64, 2048, 300
NCORES = 8
P = 128
DP = 384          # 3 k-slices of 128 (cols 300..383 zero)
KT = 3
MAXSZ = 4         # max tiles per group

bf16 = ml_dtypes.bfloat16

_cache = {}


def _build_nc(group_sizes):
    """One SPMD program for the given per-core group-size list (sorted desc)."""
    from contextlib import ExitStack
    import concourse.tile as tile
    from concourse import mybir, bacc

    f32 = mybir.dt.float32
    bf = mybir.dt.bfloat16
    AF = mybir.ActivationFunctionType
    OP = mybir.AluOpType

    G = len(group_sizes)
    total_tok = 128 * sum(group_sizes)
    PF = 3  # DMA prefetch depth (groups)

    nc = bacc.Bacc("TRN2", target_bir_lowering=False, debug=False)
    x_dram = nc.dram_tensor("x", [total_tok, DP], bf, kind="ExternalInput")
    wt_dram = nc.dram_tensor("wt", [P, KT * KT, P], bf, kind="ExternalInput")
    b_dram = nc.dram_tensor("bv", [P, KT], f32, kind="ExternalInput")
    s_dram = nc.dram_tensor("s", [P, KT, G], f32, kind="ExternalOutput")
    q_dram = nc.dram_tensor("q", [P, KT, G], f32, kind="ExternalOutput")

    offs = np.concatenate([[0], np.cumsum(np.asarray(group_sizes) * 128)])

    with ExitStack() as ctx:
        tc = ctx.enter_context(tile.TileContext(nc))
        const = ctx.enter_context(tc.tile_pool(name="const", bufs=1))
        xtp = ctx.enter_context(tc.tile_pool(name="xtp", bufs=PF + 2))
        yp = ctx.enter_context(tc.tile_pool(name="yp", bufs=3))
        y2p = ctx.enter_context(tc.tile_pool(name="y2p", bufs=3))
        ps_y = ctx.enter_context(tc.tile_pool(name="ps_y", bufs=6, space="PSUM"))
        ps_w = ctx.enter_context(tc.tile_pool(name="ps_w", bufs=1, space="PSUM"))

        wt_sb = const.tile([P, KT * KT, P], bf)
        nc.sync.dma_start(wt_sb[:], wt_dram.ap())
        b_sb = const.tile([P, KT], f32)
        nc.sync.dma_start(b_sb[:], b_dram.ap())
        sacc = const.tile([P, KT, G], f32)
        qacc = const.tile([P, KT, G], f32)

        # PE warmup: ramp HAM toward 8/8 while the first x DMAs are in flight.
        pwu = ps_w.tile([P, 2 * P], f32, name="pwu", tag="warm")
        for w in range(10):
            nc.tensor.matmul(pwu[:, (w % 2) * P:(w % 2) * P + P],
                             wt_sb[:, w % (KT * KT), :], wt_sb[:, 0, :])

        xts = {}

        def fetch(g):
            sz = group_sizes[g]
            xt = xtp.tile([P, KT, 128 * sz], bf, name=f"xt{g}", tag="xt")
            nc.sync.dma_start_transpose(
                xt[:], x_dram.ap()[offs[g]:offs[g] + 128 * sz, :])
            xts[g] = xt

        for g in range(min(PF, G)):
            fetch(g)

        for g in range(G):
            sz = group_sizes[g]
            n = 128 * sz
            if g + PF < G:
                fetch(g + PF)
            xt = xts.pop(g)
            for i in range(KT):
                py = ps_y.tile([P, n], f32, name=f"py{g}_{i}", tag="py")
                for j in range(KT):
                    nc.tensor.matmul(
                        py[:], wt_sb[:, KT * i + j, :], xt[:, j, :],
                        start=(j == 0), stop=(j == KT - 1),
                    )
                y_sb = yp.tile([P, n], bf, name=f"y{g}_{i}", tag="y")
                nc.scalar.activation(
                    y_sb[:], py[:], AF.Gelu,
                    bias=b_sb[:, i:i + 1], scale=1.0,
                    accum_out=sacc[:, i, g:g + 1],
                )
                y2_sb = y2p.tile([P, n], bf, name=f"y2{g}_{i}", tag="y2")
                nc.vector.scalar_tensor_tensor(
                    y2_sb[:], y_sb[:], 1.0, y_sb[:],
                    OP.mult, OP.mult, accum_out=qacc[:, i, g:g + 1],
                )
        nc.sync.dma_start(s_dram.ap()[:], sacc[:])
        nc.sync.dma_start(q_dram.ap()[:], qacc[:])

    nc.compile()
    return nc


def _plan(valid):
    """Build the packing schedule from the validity mask [B, T].

    Returns (group_sizes, per-core schedules).  Each schedule entry is
    (batch, tile_indices) for one group; batch < 0 marks a dummy group."""
    TPB = T // 128
    vt = valid.reshape(B, TPB, 128)
    keep = vt.any(axis=2)  # [B, TPB] tiles with >=1 valid token

    groups = []  # (size, batch, tile_idx_list)
    for b in range(B):
        tiles = np.nonzero(keep[b])[0].tolist()
        for k in range(0, len(tiles), MAXSZ):
            chunk = tiles[k:k + MAXSZ]
            groups.append((len(chunk), b, chunk))

    # pad each size class to a multiple of NCORES with dummy groups
    from collections import Counter
    cnt = Counter(g[0] for g in groups)
    for s in list(cnt):
        for _ in range((-cnt[s]) % NCORES):
            groups.append((s, -1, []))
    groups.sort(key=lambda g: -g[0])

    # deal round-robin: core c takes groups c, c+8, ... -> identical size lists
    scheds = [[] for _ in range(NCORES)]
    for idx, g in enumerate(groups):
        scheds[idx % NCORES].append(g)
    group_sizes = tuple(g[0] for g in scheds[0])
    for c in range(1, NCORES):
        assert tuple(g[0] for g in scheds[c]) == group_sizes
    return group_sizes, scheds


def _pack_inputs(x, W, b, valid, group_sizes, scheds):
    """Build per-core input maps and the slot->batch bookkeeping."""
    total_tok = 128 * sum(group_sizes)
    vt = valid.reshape(B, T // 128, 128)

    wt = np.zeros((P, KT * KT, P), np.float32)
    Wp = np.zeros((DP, DP), np.float32)
    Wp[:D, :D] = np.asarray(W, np.float32)
    for i in range(KT):
        for j in range(KT):
            wt[:, KT * i + j, :] = Wp[128 * i:128 * i + 128,
                                      128 * j:128 * j + 128].T
    wtb = wt.astype(bf16)
    bp = np.zeros(DP, np.float32)
    bp[:D] = np.asarray(b, np.float32)
    bv = np.ascontiguousarray(bp.reshape(KT, P).T)

    x32 = np.asarray(x, np.float32)
    in_maps = []
    slot_info = []  # per core: list of (batch, pad_count) per group slot
    for c in range(NCORES):
        xc = np.zeros((total_tok, DP), bf16)
        info = []
        row = 0
        for (sz, bidx, tiles) in scheds[c]:
            npad = 0
            for t in tiles:
                seg = x32[bidx, t * 128:(t + 1) * 128, :]  # [128, 300]
                v = vt[bidx, t]  # [128] bool
                blk = np.where(v[:, None], seg, 0.0).astype(bf16)
                xc[row:row + 128, :D] = blk
                npad += int(128 - v.sum())
                row += 128
            row += 128 * (sz - len(tiles))  # dummy groups stay zero
            info.append((bidx, npad + 128 * (sz - len(tiles))))
        in_maps.append({"x": xc, "wt": wtb, "bv": bv})
        slot_info.append(info)
    return in_maps, slot_info


def _host_epilogue(res, slot_info, n_valid, b):
    from scipy.special import erf

    bf32 = np.zeros(DP, np.float64)
    bf32[:D] = np.asarray(b, np.float64)
    ypad = bf32 * 0.5 * (1.0 + erf(bf32 / np.sqrt(2.0)))  # fp32-datapath gelu(b)
    ypad_sq = np.square(
        ypad.astype(np.float32).astype(bf16).astype(np.float64))

    S = np.zeros((B, DP), np.float64)
    Q = np.zeros((B, DP), np.float64)
    pad_cnt = np.zeros(B, np.int64)
    for c in range(NCORES):
        s = np.asarray(res[c]["s"], np.float64)  # [128, 3, G]
        q = np.asarray(res[c]["q"], np.float64)
        for g, (bidx, npad) in enumerate(slot_info[c]):
            if bidx < 0:
                continue
            S[bidx] += s[:, :, g].T.reshape(DP)
            Q[bidx] += q[:, :, g].T.reshape(DP)
            pad_cnt[bidx] += npad

    S = S[:, :D] - pad_cnt[:, None] * ypad[None, :D]
    Q = Q[:, :D] - pad_cnt[:, None] * ypad_sq[None, :D]
    n = n_valid.astype(np.float64)[:, None]

    with np.errstate(divide="ignore", invalid="ignore"):
        mean = S / n
        var = (Q - S * S / n) / np.maximum(n - 1.0, 1.0)
        std = np.where(n > 1.0, np.sqrt(np.maximum(var, 0.0)), 0.0)
    out = np.concatenate([std, mean], axis=-1).astype(np.float32)
    return np.where(np.isnan(out), np.float32(0.0), out)


def _prep(x, W, b):
    x32 = np.asarray(x, np.float32)
    valid = ~np.all(x32 == -1.0, axis=2)  # [B, T]
    group_sizes, scheds = _plan(valid)
    in_maps, slot_info = _pack_inputs(x, W, b, valid, group_sizes, scheds)
    return group_sizes, in_maps, slot_info, valid.sum(1)


def _trace_in_maps(ins):
    group_sizes, in_maps, _, _ = _prep(**ins)
    return in_maps


def kernel(x, W, b):
    from concourse.bass_utils import run_bass_kernel_spmd

    group_sizes, in_maps, slot_info, n_valid = _prep(x, W, b)
    if group_sizes not in _cache:
        _cache[group_sizes] = _build_nc(list(group_sizes))
        _cache["nc"] = _cache[group_sizes]  # latest, for test.py tracing
    nc = _cache[group_sizes]
    _cache["nc"] = nc

    res = run_bass_kernel_spmd(nc, in_maps, core_ids=list(range(NCORES)))
    return _host_epilogue(res.results, slot_info, n_valid, b)


# revision 7
# speedup vs baseline: 3.2552x; 1.0495x over previous
"""Trainium2 Bass kernel for nn_LinearNNEncoder (fused Linear+GELU, masked per-batch
mean/std over ragged sequences), data-parallel over 8 NeuronCores.

Contract: kernel(**inputs) takes FULL inputs (x [64,2048,300] f32, W [300,300],
b [300]) and returns the FULL output [64, 600] f32 (concat(std, mean) per batch).

Design (v3):
  - Host drops all-padding 128-token tiles (ragged lengths -> ~0.65x work), zeroes
    the remaining pad rows, and packs tiles into batch-aligned GROUPS of 1..4 tiles.
    Every core receives the same sorted multiset of group sizes (dummy all-zero
    groups pad each size class to a multiple of 8), so one SPMD program serves all
    cores while per-core data differs.
  - Per group: one dma_start_transpose loads x^T [128, 3, 128*sz] bf16 straight
    from DRAM (HW xbar transpose; d on partitions).  PE runs 9 matmuls (W-slice
    stationary [128,128] bf16, x^T moving) accumulating pre-activations per
    128-wide output slice in PSUM.  ScalarE applies exact GELU with the bias as a
    per-partition vector and emits the per-slice token-sum via accum_out (fp32
    datapath).  VectorE squares y (scalar_tensor_tensor) and emits sum(y^2) via
    accum_out.  Sums land in per-(slice, group) slots; one DMA returns them.
  - Host epilogue: sums slots per batch, subtracts the analytic contribution of
    the zeroed pad rows (gelu(b) per token; bf16-rounded for the square sum), and
    computes mean/std (unbiased, n<=1 and NaN edge cases per the reference).
"""
import numpy as np
import ml_dtypes

B, T, D = 64, 2048, 300
NCORES = 8
P = 128
DP = 384          # 3 k-slices of 128 (cols 300..383 zero)
KT = 3
MAXSZ = 4         # max tiles per group

bf16 = ml_dtypes.bfloat16

_cache = {}


def _build_nc(group_sizes):
    """One SPMD program for the given per-core group-size list (sorted desc)."""
    from contextlib import ExitStack
    import concourse.tile as tile
    from concourse import mybir, bacc

    f32 = mybir.dt.float32
    bf = mybir.dt.bfloat16
    AF = mybir.ActivationFunctionType
    OP = mybir.AluOpType

    G = len(group_sizes)
    total_tok = 128 * sum(group_sizes)
    PF = 3  # DMA prefetch depth (groups)

    nc = bacc.Bacc("TRN2", target_bir_lowering=False, debug=False)
    x_dram = nc.dram_tensor("x", [total_tok, DP], bf, kind="ExternalInput")
    wt_dram = nc.dram_tensor("wt", [P, KT * KT, P], bf, kind="ExternalInput")
    b_dram = nc.dram_tensor("bv", [P, KT], f32, kind="ExternalInput")
    bn_dram = nc.dram_tensor("bn", [P, KT, G, 6], f32, kind="ExternalOutput")

    offs = np.concatenate([[0], np.cumsum(np.asarray(group_sizes) * 128)])

    with ExitStack() as ctx:
        tc = ctx.enter_context(tile.TileContext(nc))
        const = ctx.enter_context(tc.tile_pool(name="const", bufs=1))
        xtp = ctx.enter_context(tc.tile_pool(name="xtp", bufs=PF + 2))
        yp = ctx.enter_context(tc.tile_pool(name="yp", bufs=6))
        ps_y = ctx.enter_context(tc.tile_pool(name="ps_y", bufs=6, space="PSUM"))
        ps_w = ctx.enter_context(tc.tile_pool(name="ps_w", bufs=1, space="PSUM"))

        wt_sb = const.tile([P, KT * KT, P], bf)
        nc.sync.dma_start(wt_sb[:], wt_dram.ap())
        b_sb = const.tile([P, KT], f32)
        nc.sync.dma_start(b_sb[:], b_dram.ap())
        bnacc = const.tile([P, KT, G, 6], f32)

        # PE warmup: ramp HAM toward 8/8 while the first x DMAs are in flight.
        pwu = ps_w.tile([P, 2 * P], f32, name="pwu", tag="warm")
        for w in range(10):
            nc.tensor.matmul(pwu[:, (w % 2) * P:(w % 2) * P + P],
                             wt_sb[:, w % (KT * KT), :], wt_sb[:, 0, :])

        xts = {}

        def fetch(g):
            sz = group_sizes[g]
            xt = xtp.tile([P, KT, 128 * sz], bf, name=f"xt{g}", tag="xt")
            nc.sync.dma_start_transpose(
                xt[:], x_dram.ap()[offs[g]:offs[g] + 128 * sz, :])
            xts[g] = xt

        for g in range(min(PF, G)):
            fetch(g)

        for g in range(G):
            sz = group_sizes[g]
            n = 128 * sz
            if g + PF < G:
                fetch(g + PF)
            xt = xts.pop(g)
            for i in range(KT):
                py = ps_y.tile([P, n], f32, name=f"py{g}_{i}", tag="py")
                for j in range(KT):
                    nc.tensor.matmul(
                        py[:], wt_sb[:, KT * i + j, :], xt[:, j, :],
                        start=(j == 0), stop=(j == KT - 1),
                    )
                y_sb = yp.tile([P, n], bf, name=f"y{g}_{i}", tag="y")
                nc.scalar.activation(
                    y_sb[:], py[:], AF.Gelu,
                    bias=b_sb[:, i:i + 1], scale=1.0,
                )
                nc.vector.bn_stats(bnacc[:, i, g, :], y_sb[:])
        nc.sync.dma_start(bn_dram.ap()[:], bnacc[:])

    nc.compile()
    return nc


def _plan(valid):
    """Build the packing schedule from the validity mask [B, T].

    Returns (group_sizes, per-core schedules).  Each schedule entry is
    (batch, tile_indices) for one group; batch < 0 marks a dummy group."""
    TPB = T // 128
    vt = valid.reshape(B, TPB, 128)
    keep = vt.any(axis=2)  # [B, TPB] tiles with >=1 valid token

    groups = []  # (size, batch, tile_idx_list)
    for b in range(B):
        tiles = np.nonzero(keep[b])[0].tolist()
        for k in range(0, len(tiles), MAXSZ):
            chunk = tiles[k:k + MAXSZ]
            groups.append((len(chunk), b, chunk))

    # pad each size class to a multiple of NCORES with dummy groups
    from collections import Counter
    cnt = Counter(g[0] for g in groups)
    for s in list(cnt):
        for _ in range((-cnt[s]) % NCORES):
            groups.append((s, -1, []))
    groups.sort(key=lambda g: -g[0])

    # deal round-robin: core c takes groups c, c+8, ... -> identical size lists
    scheds = [[] for _ in range(NCORES)]
    for idx, g in enumerate(groups):
        scheds[idx % NCORES].append(g)
    group_sizes = tuple(g[0] for g in scheds[0])
    for c in range(1, NCORES):
        assert tuple(g[0] for g in scheds[c]) == group_sizes
    return group_sizes, scheds


def _pack_inputs(x, W, b, valid, group_sizes, scheds):
    """Build per-core input maps and the slot->batch bookkeeping."""
    total_tok = 128 * sum(group_sizes)
    vt = valid.reshape(B, T // 128, 128)

    wt = np.zeros((P, KT * KT, P), np.float32)
    Wp = np.zeros((DP, DP), np.float32)
    Wp[:D, :D] = np.asarray(W, np.float32)
    for i in range(KT):
        for j in range(KT):
            wt[:, KT * i + j, :] = Wp[128 * i:128 * i + 128,
                                      128 * j:128 * j + 128].T
    wtb = wt.astype(bf16)
    bp = np.zeros(DP, np.float32)
    bp[:D] = np.asarray(b, np.float32)
    bv = np.ascontiguousarray(bp.reshape(KT, P).T)

    x32 = np.asarray(x, np.float32)
    in_maps = []
    slot_info = []  # per core: list of (batch, pad_count) per group slot
    for c in range(NCORES):
        xc = np.zeros((total_tok, DP), bf16)
        info = []
        row = 0
        for (sz, bidx, tiles) in scheds[c]:
            npad = 0
            for t in tiles:
                seg = x32[bidx, t * 128:(t + 1) * 128, :]  # [128, 300]
                v = vt[bidx, t]  # [128] bool
                blk = np.where(v[:, None], seg, 0.0).astype(bf16)
                xc[row:row + 128, :D] = blk
                npad += int(128 - v.sum())
                row += 128
            row += 128 * (sz - len(tiles))  # dummy groups stay zero
            info.append((bidx, npad + 128 * (sz - len(tiles))))
        in_maps.append({"x": xc, "wt": wtb, "bv": bv})
        slot_info.append(info)
    return in_maps, slot_info


def _host_epilogue(res, slot_info, n_valid, b):
    from scipy.special import erf

    bf32 = np.zeros(DP, np.float64)
    bf32[:D] = np.asarray(b, np.float64)
    # pad rows: psum 0 -> y = bf16(gelu(b)); bn_stats reads the bf16 y
    ypad = bf32 * 0.5 * (1.0 + erf(bf32 / np.sqrt(2.0)))
    ypad = ypad.astype(np.float32).astype(bf16).astype(np.float64)
    ypad_sq = np.square(ypad)

    S = np.zeros((B, DP), np.float64)
    Q = np.zeros((B, DP), np.float64)
    pad_cnt = np.zeros(B, np.int64)
    for c in range(NCORES):
        bn = np.asarray(res[c]["bn"], np.float64)  # [128, 3, G, 6]
        s_all = bn[..., 0] * bn[..., 1] + bn[..., 3] * bn[..., 4]
        q_all = (bn[..., 2] + bn[..., 0] * np.square(bn[..., 1])
                 + bn[..., 5] + bn[..., 3] * np.square(bn[..., 4]))
        for g, (bidx, npad) in enumerate(slot_info[c]):
            if bidx < 0:
                continue
            S[bidx] += s_all[:, :, g].T.reshape(DP)
            Q[bidx] += q_all[:, :, g].T.reshape(DP)
            pad_cnt[bidx] += npad

    S = S[:, :D] - pad_cnt[:, None] * ypad[None, :D]
    Q = Q[:, :D] - pad_cnt[:, None] * ypad_sq[None, :D]
    n = n_valid.astype(np.float64)[:, None]

    with np.errstate(divide="ignore", invalid="ignore"):
        mean = S / n
        var = (Q - S * S / n) / np.maximum(n - 1.0, 1.0)
        std = np.where(n > 1.0, np.sqrt(np.maximum(var, 0.0)), 0.0)
    out = np.concatenate([std, mean], axis=-1).astype(np.float32)
    return np.where(np.isnan(out), np.float32(0.0), out)


def _prep(x, W, b):
    x32 = np.asarray(x, np.float32)
    valid = ~np.all(x32 == -1.0, axis=2)  # [B, T]
    group_sizes, scheds = _plan(valid)
    in_maps, slot_info = _pack_inputs(x, W, b, valid, group_sizes, scheds)
    return group_sizes, in_maps, slot_info, valid.sum(1)


def _trace_in_maps(ins):
    group_sizes, in_maps, _, _ = _prep(**ins)
    return in_maps


def kernel(x, W, b):
    from concourse.bass_utils import run_bass_kernel_spmd

    group_sizes, in_maps, slot_info, n_valid = _prep(x, W, b)
    if group_sizes not in _cache:
        _cache[group_sizes] = _build_nc(list(group_sizes))
        _cache["nc"] = _cache[group_sizes]  # latest, for test.py tracing
    nc = _cache[group_sizes]
    _cache["nc"] = nc

    res = run_bass_kernel_spmd(nc, in_maps, core_ids=list(range(NCORES)))
    return _host_epilogue(res.results, slot_info, n_valid, b)
